# revision 14
# baseline (speedup 1.0000x reference)
"""Trainium2 Bass kernel for Enformer-style relative-position attention.

Problem: nn_Attention_79087527788690
  x [1, 2048, 1536] -> out [1, 2048, 1536]
  8 heads, dk=64, dv=192, rel-pos features=192, n=2048.

Sharding: one head per NeuronCore (8 cores). Each core computes its head's
q/k/v projections, content + relative-position logits, softmax weights and
per-head attention output oh [2048, 192]. Transposed oh row-tiles are
exchanged via two AllToAll collectives (tiles 0-7, then 8-15; core c owns
tiles {c, 8+c}); each core then multiplies its two owned row-tiles against
the full (dv-swizzled) Wo for its final [256, 1536] output rows.

Transposed-native pipeline (exp-split):
- The rel-logit window R[i, u] is exp'd during PSUM evacuation (ACT),
  written to DRAM, and read back through the DMA xbar TRANSPOSE over the
  skewed access pattern: relT[p, b, i] = exp(R)_shifted[i, 128b+p] - the
  shear and the transpose ride one mandatory DMA round trip.
- Content logits are computed already transposed (kT stationary per j-tile,
  qcT moving); ACT exps the f32 PSUM -> E0^T; DVE multiplies by relT in
  4x mode -> E^T tiles [128j, 16, 128i] (bf16).
- PV consumes E^T blocks directly as matmul stationaries (no PE transposes);
  a ones-column in v accumulates the softmax row sums in the same PSUM.

Stage leads per iteration it: G matmuls for it+4, gd write for it+3,
transposed read for it+2, logits for it, PV for it-K_TRAIL - every DMA's
producer is >=1 iteration old, so the in-order DMA queues never head-block.
Bulk loads (x slices, Wo) are chunked and drip-fed between stage DMAs.

relk (= positional_embed @ Wrel, weight-only, independent of x) is
precomputed host-side per head and fed replicated, per the sharding hint.
"""

import math
import os
import sys
from contextlib import ExitStack

sys.path.insert(0, "/opt/trn_rl_repo")

import numpy as np

N = 2048
DIM = 1536
HEADS = 8
DK = 64
DV = 192
F = 192  # rel pos features
SPAN = 2 * N - 1  # 4095
NCORES = 8
CHUNK = N // NCORES  # 256
SCALE = DK ** -0.5

K_TRAIL = int(os.environ.get("K_TRAIL", "2"))
K_EB = int(os.environ.get("K_EB", "4"))
K_RB = int(os.environ.get("K_RB", "4"))
K_FB = int(os.environ.get("K_FB", "2"))
K_PGB = int(os.environ.get("K_PGB", "4"))   # G window psum ring ([128,1024] f16)
K_PCB = int(os.environ.get("K_PCB", "2"))   # content psum ring
K_POB = int(os.environ.get("K_POB", "1"))
K_FINE = os.environ.get("K_FINE", "dve")    # outproj psum evacuation engine
K_QCE = os.environ.get("K_QCE", "act")      # qc/qp bias evacuation engine
K_OP0 = os.environ.get("K_OP0", "steady")
K_COMM0 = int(os.environ.get("K_COMM0", "11"))
K_OP0IT = int(os.environ.get("K_OP0IT", "12"))
K_OUT16 = os.environ.get("K_OUT16", "1") == "1"
K_ORDER = os.environ.get("K_ORDER", "cgp")  # per-iter PE stage order

GL_MM = 4   # G matmul lead
GL_WR = 3   # gd write lead
GL_RD = 2   # relT read lead

IT = 128          # q rows per tile
NIT = N // IT     # 16
JC = 512          # j chunk for logits
NJC = N // JC     # 4
GW = N + IT - 1   # 2175, G window per i-tile
GPITCH = 2176     # padded pitch of the DRAM G buffer
NAG = 2           # all-to-all groups (tiles 0-7, 8-15)


def _positions() -> np.ndarray:
    """get_positional_embed(2048, 192) in numpy (f64 -> f32). [4095, 192]"""
    d = np.arange(-N + 1, N).astype(np.float64)
    nb = F // 6
    absd = np.abs(d)[:, None]
    max_range = math.log(N) / math.log(2.0)
    half_life = 2.0 ** np.linspace(3.0, max_range, nb)
    feat_exp = np.exp(-math.log(2.0) / half_life[None, :] * absd)
    cw = 2.0 ** np.arange(1, nb + 1) - 1.0
    feat_cm = (cw[None, :] > absd).astype(np.float64)
    stddev = N / (2 * nb)
    start_mean = N / nb
    mean = np.linspace(start_mean, N, nb)[None, :]
    conc = (mean / stddev) ** 2
    rate = mean / stddev ** 2
    with np.errstate(divide="ignore", invalid="ignore"):
        log_unnorm = (conc - 1.0) * np.log(absd) - rate * absd
    log_unnorm = np.where(absd == 0, -np.inf, log_unnorm)
    lg = np.vectorize(math.lgamma)(conc)
    log_norm = lg - conc * np.log(rate)
    probs = np.exp(log_unnorm - log_norm) + 1e-8
    feat_gamma = probs / np.amax(probs, axis=-1, keepdims=True)
    emb = np.concatenate([feat_exp, feat_cm, feat_gamma], axis=-1)
    out = np.concatenate([emb, np.sign(d)[:, None] * emb], axis=-1)
    return out.astype(np.float32)


def build_nc(num_cores: int = NCORES, collective: bool = True):
    """Build + compile the per-core Bass graph (SPMD, identical on all cores)."""
    import concourse.bass as bass
    import concourse.mybir as mybir
    import concourse.tile as tile
    from concourse import bacc
    from concourse.masks import make_identity

    f32 = mybir.dt.float32
    f16 = mybir.dt.float16
    bf16 = mybir.dt.bfloat16
    ODT = f16 if K_OUT16 else f32

    nc = bacc.Bacc(
        "TRN2", target_bir_lowering=False, debug=False, num_devices=num_cores
    )

    # --- external I/O (per-core shards supplied via in_maps) ---
    xT_e = nc.dram_tensor("xT", [DIM, N], f16, kind="ExternalInput")
    wqk_e = nc.dram_tensor("wqk", [128, 12, 2 * DK], f16, kind="ExternalInput")
    wvp_e = nc.dram_tensor("wvp", [128, 12, DV], f16, kind="ExternalInput")
    relkT_e = nc.dram_tensor("relkT", [DK, SPAN], f16, kind="ExternalInput")
    rb2_e = nc.dram_tensor("rb2", [DK, 2], f32, kind="ExternalInput")
    wof_e = nc.dram_tensor("wof", [128, 12, DIM], f16, kind="ExternalInput")
    bo_e = nc.dram_tensor("bo", [1, DIM], f16, kind="ExternalInput")
    out_e = nc.dram_tensor("out", [CHUNK, DIM], ODT, kind="ExternalOutput")
    K_DBG = os.environ.get("K_DBG", "0") == "1"
    if K_DBG:
        dbg_kT = nc.dram_tensor("dbg_kT", [DK, N], f16, kind="ExternalOutput")
        dbg_qcT = nc.dram_tensor("dbg_qcT", [DK, N], f16, kind="ExternalOutput")
        dbg_vb = nc.dram_tensor("dbg_vb", [128, NIT, DV + 1], bf16, kind="ExternalOutput")
        dbg_ET = nc.dram_tensor("dbg_ET", [128, NIT, IT], bf16, kind="ExternalOutput")
        dbg_relT = nc.dram_tensor("dbg_relT", [128, NIT, IT], bf16, kind="ExternalOutput")


    # --- internal DRAM ---
    a2a_ins = [nc.dram_tensor(f"a2a_in{g}", [NCORES, DV, IT], f16) for g in range(NAG)]
    a2a_outs = [nc.dram_tensor(f"a2a_out{g}", [NCORES, DV, IT], f16) for g in range(NAG)]
    NGD = 5
    gds = [nc.dram_tensor(f"gd{i}", [IT, GPITCH], bf16) for i in range(NGD)]

    with tile.TileContext(nc) as tc, ExitStack() as ctx:
        const = ctx.enter_context(tc.tile_pool(name="const", bufs=1))
        work = ctx.enter_context(tc.tile_pool(name="work", bufs=2))
        psum = ctx.enter_context(tc.tile_pool(name="psum", bufs=2, space="PSUM"))

        ident_h = const.tile([128, 128], f16, tag="idh")
        make_identity(nc, ident_h[:])
        ones_r = const.tile([1, 128], f16, tag="onesr")
        nc.vector.memset(ones_r[:], 1.0)

        xs = [const.tile([128, 12, 512], f16, tag=f"xs{sl}", name=f"xs{sl}")
              for sl in range(4)]
        wqk_s = const.tile([128, 12, 2 * DK], f16, tag="wqk")
        wv_s = const.tile([128, 12, DV], f16, tag="wv")
        relkT = const.tile([DK, SPAN + 1], f16, tag="relkT")
        rb2_s = const.tile([DK, 2], f32, tag="rb2")
        bo_r = const.tile([1, DIM], f16, tag="bor")
        wof_p = [const.tile([128, 4, DIM], f16, tag=f"wof{j}", name=f"wof{j}")
                 for j in range(3)]

        # startup-critical loads
        nc.sync.dma_start(out=wqk_s[:], in_=wqk_e[:])
        nc.sync.dma_start(out=rb2_s[:], in_=rb2_e[:])
        nc.sync.dma_start(out=relkT[:, 0:SPAN], in_=relkT_e[:])
        for c in range(12):
            nc.sync.dma_start(out=xs[0][:, c, :],
                              in_=xT_e[128 * c:128 * (c + 1), 0:512])
        nc.sync.dma_start(out=wv_s[:], in_=wvp_e[:])

        # bulk-load filler queues: chunked so no single long transfer parks
        # the serial DMA engine. ensure_xs(sl) flushes a slice's remaining
        # chunks BEFORE its first consumer is emitted (emission order defines
        # the dependency order).
        xs_fill = {sl: [(xs[sl][:, c, :],
                         bass.AP(xT_e, 512 * sl + N * 128 * c,
                                 [[N, 128], [1, 512]]))
                        for c in range(12)] for sl in (1, 2, 3)}
        w_fill = [(bo_r[:], bo_e[:])]
        for j in range(3):
            for half in range(2):
                w_fill.append((wof_p[j][:, 2 * half:2 * half + 2, :],
                               wof_e[:, 4 * j + 2 * half:4 * j + 2 * half + 2, :]))

        def emit_fillers(k):
            for _ in range(k):
                for sl in (1, 2, 3):
                    if xs_fill[sl]:
                        o, i = xs_fill[sl].pop(0)
                        nc.sync.dma_start(out=o, in_=i)
                        break
                else:
                    if w_fill:
                        o, i = w_fill.pop(0)
                        nc.sync.dma_start(out=o, in_=i)

        def ensure_xs(sl):
            while xs_fill.get(sl):
                o, i = xs_fill[sl].pop(0)
                nc.sync.dma_start(out=o, in_=i)

        def ensure_w():
            while w_fill:
                o, i = w_fill.pop(0)
                nc.sync.dma_start(out=o, in_=i)

        # ---- projections ----
        qcT = const.tile([DK, N], f16, tag="qcT")
        qpT = const.tile([DK, N], f16, tag="qpT")
        kT = const.tile([DK, N], f16, tag="kT")
        vb = const.tile([128, NIT, DV + 1], bf16, tag="vb")
        nc.vector.memset(vb[:, :, DV:DV + 1], 1.0)

        def emit_qk(ic):
            pq = psum.tile([128, 512], f32, tag="pc", bufs=K_PCB, name=f"pq{ic}")
            for c in range(12):
                nc.tensor.matmul(pq[:], wqk_s[:, c, :],
                                 xs[ic][:, c, :],
                                 start=(c == 0), stop=(c == 11))
            if K_QCE == "dve":
                nc.vector.tensor_scalar(qcT[:, 512 * ic:512 * (ic + 1)],
                                        pq[0:DK, :], rb2_s[:, 0:1], None,
                                        mybir.AluOpType.add)
                nc.vector.tensor_scalar(qpT[:, 512 * ic:512 * (ic + 1)],
                                        pq[0:DK, :], rb2_s[:, 1:2], None,
                                        mybir.AluOpType.add)
            else:
                nc.scalar.activation(qcT[:, 512 * ic:512 * (ic + 1)], pq[0:DK, :],
                                     mybir.ActivationFunctionType.Identity,
                                     bias=rb2_s[:, 0:1], scale=1.0)
                nc.scalar.activation(qpT[:, 512 * ic:512 * (ic + 1)], pq[0:DK, :],
                                     mybir.ActivationFunctionType.Identity,
                                     bias=rb2_s[:, 1:2], scale=1.0)
            nc.vector.tensor_copy(kT[:, 512 * ic:512 * (ic + 1)], pq[DK:2 * DK, :])

        def emit_v(jt):
            pv = psum.tile([128, DV], f32, tag="po", bufs=K_POB, name=f"pv{jt}")
            for c in range(12):
                nc.tensor.matmul(pv[:], xs[jt // 4][:, c, IT * (jt % 4):IT * (jt % 4 + 1)],
                                 wv_s[:, c, :], start=(c == 0), stop=(c == 11))
            nc.vector.tensor_copy(vb[:, jt, 0:DV], pv[:])

        # ---- G stage: rel-logit window matmuls + exp evacuation (ACT) into
        # gwin, then DRAM write + sheared-transposed read-back. ----
        g_win = {}

        def emit_g_mm(it):
            i0 = IT * it
            w0 = (N - IT) - i0
            gwin = work.tile([128, GPITCH], bf16, tag="gwin", bufs=K_PGB,
                             name=f"gwin{it}")
            for q in range(4):
                pg = psum.tile([128, JC], f32, tag="pg", bufs=2,
                               name=f"pg{it}_{q}")
                nc.tensor.matmul(pg[:], qpT[:, i0:i0 + IT],
                                 relkT[:, w0 + JC * q:w0 + JC * (q + 1)],
                                 start=True, stop=True)
                nc.scalar.activation(gwin[:, JC * q:JC * (q + 1)], pg[:],
                                     mybir.ActivationFunctionType.Exp)
            pg2 = psum.tile([128, IT], f32, tag="pp", bufs=2, name=f"pg2_{it}")
            nc.tensor.matmul(pg2[:, 0:IT - 1], qpT[:, i0:i0 + IT],
                             relkT[:, w0 + 4 * JC:w0 + GW], start=True, stop=True)
            nc.scalar.activation(gwin[:, 4 * JC:GW], pg2[:, 0:IT - 1],
                                 mybir.ActivationFunctionType.Exp)
            g_win[it] = gwin

        def emit_g_write(it):
            gwin = g_win.pop(it)
            gd = gds[it % NGD]
            nc.sync.dma_start(out=gd[:, 0:GW], in_=gwin[:, 0:GW])

        def emit_g_read(it):
            gd = gds[it % NGD]
            diag = bass.AP(gd, 127, [[GW, 128], [1, N]])
            relT = work.tile([128, NIT, IT], bf16, tag="relT", bufs=K_RB,
                             name=f"relT{it}")
            nc.sync.dma_start_transpose(out=relT[:], in_=diag)
            return relT

        # ---- content logits (transposed) + exp + E^T = expC^T * expR^T ----
        def emit_logits(it, relT):
            i0 = IT * it
            ET = work.tile([128, NIT, IT], bf16, tag="E", bufs=K_EB, name=f"E{it}")
            for jq in range(NJC):
                pcT = psum.tile([128, 4, IT], f32, tag="pc", bufs=K_PCB,
                                name=f"pcT{it}_{jq}")
                for q in range(4):
                    jt = 4 * jq + q
                    nc.tensor.matmul(pcT[:, q, :], kT[:, IT * jt:IT * (jt + 1)],
                                     qcT[:, i0:i0 + IT], start=True, stop=True)
                E0T = work.tile([128, 4, IT], bf16, tag="E0", bufs=3,
                                name=f"E0_{it}_{jq}")
                nc.scalar.activation(E0T[:], pcT[:],
                                     mybir.ActivationFunctionType.Exp)
                nc.vector.tensor_tensor(ET[:, 4 * jq:4 * (jq + 1), :], E0T[:],
                                        relT[:, 4 * jq:4 * (jq + 1), :],
                                        mybir.AluOpType.mult)
            if K_DBG and it == 0:
                nc.sync.dma_start(out=dbg_ET[:], in_=ET[:])
                nc.sync.dma_start(out=dbg_relT[:], in_=relT[:])
            return (ET,)

        # ---- PV + rowsum column + ohT (send deferred to next iteration) ----
        snd_q = []

        def emit_pv(it, ET):
            po = psum.tile([128, DV + 1], f32, tag="po", bufs=K_POB, name=f"po{it}")
            for jt in range(NIT):
                nc.tensor.matmul(po[:], ET[:, jt, :], vb[:, jt, :],
                                 start=(jt == 0), stop=(jt == NIT - 1))
            rcp = work.tile([128, 1], f32, tag="rcp", bufs=2, name=f"rcp{it}")
            nc.vector.reciprocal(rcp[:], po[:, DV:DV + 1])
            oh = work.tile([128, DV], f16, tag="oh", name=f"oh{it}")
            nc.vector.tensor_scalar(oh[:], po[:, 0:DV], rcp[:], None,
                                    mybir.AluOpType.mult)
            ohT = work.tile([96, 2, 128], f16, tag="ohT", bufs=3, name=f"ohT{it}")
            for h in range(2):
                pth = psum.tile([96, 128], f16, tag="pt", bufs=1,
                                name=f"pth{it}_{h}")
                nc.tensor.transpose(pth[:], oh[:, 96 * h:96 * (h + 1)], ident_h[:])
                nc.vector.tensor_copy(ohT[:, h, :], pth[:])
            snd_q.append((it, ohT))

        def emit_send():
            if snd_q:
                it, ohT = snd_q.pop(0)
                ag = it // 8
                nc.sync.dma_start(
                    out=bass.AP(a2a_ins[ag], (it % 8) * DV * IT,
                                [[IT, 96], [96 * IT, 2], [1, IT]]),
                    in_=ohT[:])

        def emit_a2a_comm(ag):
            if collective:
                nc.gpsimd.collective_compute(
                    "AllToAll",
                    mybir.AluOpType.bypass,
                    replica_groups=[list(range(num_cores))],
                    ins=[a2a_ins[ag][:]],
                    outs=[a2a_outs[ag][:]],
                )
                cc_src = a2a_outs[ag]
            else:
                cc_src = a2a_ins[ag]  # timing mirror: same local read traffic
            agb = work.tile([128, 12, IT], f16, tag="agb", bufs=2, name=f"agb{ag}")
            HB = DV * IT
            nc.sync.dma_start(
                out=agb[:, 0:8, :],
                in_=bass.AP(cc_src, 0, [[IT, 128], [HB, 8], [1, IT]]))
            for b in range(2):
                nc.sync.dma_start(
                    out=agb[64 * b:64 * (b + 1), 8:12, :],
                    in_=bass.AP(cc_src, b * HB + 128 * IT,
                                [[IT, 64], [2 * HB, 4], [1, IT]]))
            return agb

        def emit_outproj(ag, agb, tag, bufs):
            fin = work.tile([128, 3, JC], ODT, tag="fin", bufs=K_FB, name=f"fin{ag}")
            for cc in range(3):
                pp = psum.tile([128, JC], f32, tag=tag, bufs=bufs,
                               name=f"ppo{ag}_{cc}")
                nc.tensor.matmul(pp[:], ones_r[:, 0:128],
                                 bo_r[:, JC * cc:JC * (cc + 1)],
                                 start=True, stop=False)
                for kc in range(12):
                    nc.tensor.matmul(pp[:], agb[:, kc, :],
                                     wof_p[kc // 4][:, kc % 4, JC * cc:JC * (cc + 1)],
                                     start=False, stop=(kc == 11))
                if K_FINE == "act":
                    nc.scalar.copy(fin[:, cc, :], pp[:])
                else:
                    nc.vector.tensor_copy(fin[:, cc, :], pp[:])
                nc.sync.dma_start(out=out_e[IT * ag:IT * (ag + 1),
                                            JC * cc:JC * (cc + 1)],
                                  in_=fin[:, cc, :])

        # ---- drive ----
        # Pre-loop: qk(0) -> G lead tiles 0..3 with their writes/reads
        # interleaved; v/qk in x-slice arrival order with chunked fillers.
        emit_qk(0)
        emit_g_mm(0)
        emit_g_mm(1)
        emit_g_write(0)
        emit_fillers(3)
        emit_g_mm(2)
        emit_g_write(1)
        rel_q = [emit_g_read(0)]
        emit_fillers(3)
        emit_g_mm(3)
        emit_g_write(2)
        rel_q.append(emit_g_read(1))
        emit_fillers(3)
        for jt in range(4):
            emit_v(jt)
        for ic in range(1, 4):
            ensure_xs(ic)
            emit_qk(ic)
            emit_fillers(2)
            for jt in range(4 * ic, 4 * ic + 4):
                emit_v(jt)
                emit_fillers(1)

        pv_q = []
        agb0 = None
        for it in range(NIT):
            # stage order on PE: content(it), G(it+GL_MM), PV(it-K_TRAIL)
            if it + GL_MM < NIT:
                emit_g_mm(it + GL_MM)
            if K_ORDER == "cgp":
                pv_q.append((it, emit_logits(it, rel_q.pop(0))))
                if len(pv_q) > K_TRAIL:
                    itp, eo = pv_q.pop(0)
                    emit_pv(itp, *eo)
            else:  # pv first
                if len(pv_q) >= K_TRAIL:
                    itp, eo = pv_q.pop(0)
                    emit_pv(itp, *eo)
                pv_q.append((it, emit_logits(it, rel_q.pop(0))))
            # DMA stages (all producers >= 1 iteration old)
            if it + GL_WR < NIT:
                emit_g_write(it + GL_WR)
            if it + GL_RD < NIT:
                rel_q.append(emit_g_read(it + GL_RD))
            emit_send()
            emit_fillers(4)
            if it == K_COMM0:
                agb0 = emit_a2a_comm(0)
            if it == K_OP0IT and K_OP0 == "steady":
                ensure_w()
                emit_outproj(0, agb0, "pg", 2)
        if K_DBG:
            nc.sync.dma_start(out=dbg_kT[:], in_=kT[:, 0:N])
            nc.sync.dma_start(out=dbg_qcT[:], in_=qcT[:, 0:N])
            nc.sync.dma_start(out=dbg_vb[:], in_=vb[:])
        ensure_w()
        for itp, eo in pv_q:
            emit_pv(itp, *eo)
        while snd_q:
            emit_send()
        agb1 = emit_a2a_comm(1)
        if K_OP0 == "drain":
            emit_outproj(0, agb0, "pg", 2)
        emit_outproj(1, agb1, "pg", 2)

    nc.compile()
    return nc


_CACHE: dict = {}


def _get_nc():
    if "nc" not in _CACHE:
        _CACHE["nc"] = build_nc()
    return _CACHE["nc"]


def _shard_inputs(x, Wq, Wk, Wv, Wrel, rel_content_bias, rel_pos_bias, Wo, bo):
    positions = _positions()  # [4095, 192] f32
    relk_all = positions @ np.asarray(Wrel, np.float32)  # [4095, 8*64]
    xT = np.ascontiguousarray(
        np.asarray(x, np.float32).reshape(N, DIM).T).astype(np.float16)
    woP = np.empty((DIM, DIM), np.float32)
    for h in range(8):
        woP[128 * h:128 * (h + 1)] = Wo[DV * h:DV * h + 128]
    for k in range(4):
        woP[1024 + 128 * k:1024 + 128 * k + 64] = Wo[DV * 2 * k + 128:DV * 2 * k + DV]
        woP[1024 + 128 * k + 64:1024 + 128 * (k + 1)] = \
            Wo[DV * (2 * k + 1) + 128:DV * (2 * k + 1) + DV]
    wof = np.ascontiguousarray(
        woP.reshape(12, 128, DIM).transpose(1, 0, 2)).astype(np.float16)
    bo_row = np.asarray(bo, np.float16).reshape(1, DIM)
    in_maps = []
    for h in range(NCORES):
        wq = (Wq[:, DK * h:DK * (h + 1)] * SCALE).astype(np.float16)
        wk = Wk[:, DK * h:DK * (h + 1)].astype(np.float16)
        wqk = np.concatenate(
            [wq.reshape(12, 128, DK), wk.reshape(12, 128, DK)], axis=2)
        wvp = Wv[:, DV * h:DV * (h + 1)].astype(np.float16).reshape(12, 128, DV)
        relkT_h = np.ascontiguousarray(
            relk_all[:, DK * h:DK * (h + 1)].T).astype(np.float16)
        rb2 = np.stack([rel_content_bias[0, h, 0, :],
                        rel_pos_bias[0, h, 0, :]], axis=1).astype(np.float32)
        in_maps.append({
            "xT": xT,
            "wqk": np.ascontiguousarray(wqk.transpose(1, 0, 2)),
            "wvp": np.ascontiguousarray(wvp.transpose(1, 0, 2)),
            "relkT": relkT_h,
            "rb2": np.ascontiguousarray(rb2),
            "wof": wof,
            "bo": bo_row,
        })
    return in_maps


def kernel(**inputs) -> np.ndarray:
    from concourse.bass_utils import run_bass_kernel_spmd

    inputs = {k: np.asarray(v) for k, v in inputs.items()}
    nc = _get_nc()
    in_maps = _shard_inputs(**inputs)
    res = run_bass_kernel_spmd(nc, in_maps, list(range(NCORES)))
    out = np.empty((N, DIM), np.float32)
    for c in range(NCORES):
        oc = np.asarray(res.results[c]["out"]).astype(np.float32)
        out[IT * c:IT * (c + 1), :] = oc[0:IT, :]
        out[1024 + IT * c:1024 + IT * (c + 1), :] = oc[IT:2 * IT, :]
    return out.reshape(1, N, DIM)
